# revision 18
# baseline (speedup 1.0000x reference)
"""Trainium2 Bass kernel for LlamaFlashAttentionMasked (EAGLE3 suffix-block attention).

Sharding: 8 cores = batch(2) x head-group(4). Each core handles 1 batch and
8 q-heads / 2 kv-heads. Per-core partial outputs (after Wo on the core's head
slice) are summed across the 4 head-groups on the host.

All-bf16 matmuls; causal mask folded into the score matmul group on the PE
(neg-identity stationary x shifted-triangle moving); 1024-wide PSUM exp
groups; suffix denominators folded into the ones-matmul; suffix q*k products
on DVE bf16; K/V projected first so each head's attention interleaves with
the remaining Q projections; each attention block's PV/denominator groups are
emitted between the NEXT block's score groups so the PE never waits on the
Act engine's exp latency; queries >= valid_seq_len are skipped.
"""
import sys
sys.path.insert(0, "/opt/trn_rl_repo")

from contextlib import ExitStack, contextmanager

import numpy as np
import ml_dtypes

import concourse.bacc as bacc
import concourse.tile as tile
import concourse.mybir as mybir
from concourse.bass_utils import run_bass_kernel_spmd
from concourse.masks import make_identity

F32 = mybir.dt.float32
BF16 = mybir.dt.bfloat16
Exp = mybir.ActivationFunctionType.Exp

HIDDEN = 4096
S = 1024
NH = 8        # q heads per core
NKV = 2       # kv heads per core
D = 128
LCK = 3
FCH = HIDDEN // 128   # 32 f-chunks
DC = NKV + NKV + NH   # 12 projection output chunks: 2 K, 2 V, 8 Q
SCALE = 1.0 / np.sqrt(D)
NEG = -1e30


def tc_ctx(nc):
    @contextmanager
    def _cm():
        with tile.TileContext(nc) as tc:
            with ExitStack() as ctx:
                yield tc, ctx
    return _cm()


def _build(qhi=1024):
    """qhi: number of live query rows (valid_seq_len clamped to (512, 1024])."""
    w1 = qhi - 512  # width of the second query half

    nc = bacc.Bacc("TRN2", target_bir_lowering=False, debug=False, num_devices=8)

    hT_d = nc.dram_tensor("hT", [FCH, 128, S], BF16, kind="ExternalInput").ap()
    w1_d = nc.dram_tensor("w1", [DC, 128, FCH, 128], BF16, kind="ExternalInput").ap()
    cos_d = nc.dram_tensor("cosT", [128, S], BF16, kind="ExternalInput").ap()
    sin_d = nc.dram_tensor("sinT", [128, S], BF16, kind="ExternalInput").ap()
    ks_d = nc.dram_tensor("ksT", [NKV, LCK, 128, S], BF16, kind="ExternalInput").ap()
    vs_d = nc.dram_tensor("vsT", [NKV, LCK, 128, S], BF16, kind="ExternalInput").ap()
    wo_d = nc.dram_tensor("wo", [NH, 128, HIDDEN], BF16, kind="ExternalInput").ap()
    out_d = nc.dram_tensor("out", [S, HIDDEN], F32, kind="ExternalOutput").ap()

    with tc_ctx(nc) as (tc, ctx):
        pers = ctx.enter_context(tc.tile_pool(name="pers", bufs=1))
        qt = pers.tile([128, NH, S], BF16, tag="qt")        # roped Q^T per head
        kt = pers.tile([128, NKV, S], BF16, tag="kt")       # roped K^T per kv head
        vn = pers.tile([128, NKV, 8, D], BF16, tag="vn")    # V natural [s-part, kv, s-chunk, d]
        ot = pers.tile([128, NH, S], BF16, tag="ot")        # normalized attn out (lhsT for Wo)
        ksT = pers.tile([128, NKV, LCK, S], BF16, tag="ksT")
        vsT = pers.tile([128, NKV, LCK, S], BF16, tag="vsT")
        cosT = pers.tile([128, S], BF16, tag="cos")
        sinT = pers.tile([128, S], BF16, tag="sin")
        ones = pers.tile([128, 128], BF16, tag="ones")
        nc.vector.memset(ones, 1.0)
        oneo = pers.tile([128, 128], BF16, tag="oneo")      # 1/128 for replicated colsums
        nc.vector.memset(oneo, 1.0 / 128.0)
        ident = pers.tile([128, 128], F32, tag="ident")
        make_identity(nc, ident)
        negid = pers.tile([128, 128], BF16, tag="negid")    # -1e30 * I
        nc.vector.tensor_scalar_mul(negid, ident, NEG)
        # ubig[p, j] = 1 if j < p + 512 else 0  (shifted causal triangle)
        ubig = pers.tile([128, 640], BF16, tag="ubig")
        nc.gpsimd.memset(ubig, 1.0)
        nc.gpsimd.affine_select(
            out=ubig, in_=ubig,
            compare_op=mybir.AluOpType.is_ge,
            fill=0.0, base=511,
            pattern=[[-1, 640]], channel_multiplier=1,
        )
        if qhi < 1024:
            # dropped query rows: zero so suffix products and phase C read
            # well-defined values
            nc.vector.memset(qt[:, :, qhi:1024], 0.0)
            nc.vector.memset(ot[:, :, qhi:1024], 0.0)

        # B-phase temp pool (opened before the A pool so closing A frees space
        # for the C pool)
        bp = ctx.enter_context(tc.tile_pool(name="bp", bufs=1))
        # single shared PSUM pool: ps(2) + stg(4) + sm(1) + otp(1) = 8 banks
        pp = ctx.enter_context(tc.tile_pool(name="pp", bufs=1, space="PSUM"))
        # initialize both stg buffers: trapezoidal score tiles leave their
        # fully-masked column ranges unwritten, and the group exp reads them
        for _ in range(2):
            stg0 = pp.tile([128, 1024], F32, tag="stg", bufs=2)
            nc.vector.memset(stg0, 0.0)

        def qwidth(qh):
            return 512 if qh == 0 else w1

        def out_pv(state):
            h, qh, kv, qlo, qw, nki, pts, pst01, pst2 = state
            otp = pp.tile([128, 512], F32, tag="otp", bufs=1)
            for ki in range(nki):
                dd = max(0, ki * 128 - qlo)
                nc.tensor.matmul(otp[:, dd:qw], vn[:, kv, ki, :],
                                 pts[ki // 2][:, (ki % 2) * 512 + dd:(ki % 2) * 512 + qw],
                                 start=(ki == 0), stop=(ki == nki - 1))
            state.append(otp)

        def out_sm(state):
            h, qh, kv, qlo, qw, nki, pts, pst01, pst2, otp = state
            # denominator: replicated suffix exps scaled by 1/128 first, then
            # colsums of the exp'd causal tiles
            sm = pp.tile([128, 512], F32, tag="sm", bufs=1)
            nc.tensor.matmul(sm, oneo, pst01[:, 0:512], start=True, stop=False)
            nc.tensor.matmul(sm, oneo, pst01[:, 512:1024], start=False, stop=False)
            nc.tensor.matmul(sm, oneo, pst2, start=False, stop=False)
            for ki in range(nki):
                dd = max(0, ki * 128 - qlo)
                nc.tensor.matmul(sm[:, dd:qw], ones,
                                 pts[ki // 2][:, (ki % 2) * 512 + dd:(ki % 2) * 512 + qw],
                                 start=False, stop=(ki == nki - 1))
            r = bp.tile([128, 512], F32, tag="r", bufs=2)
            nc.vector.reciprocal_approx_fast(out=r, in_=sm)
            state.append(r)

        def out_combine(state):
            h, qh, kv, qlo, qw, nki, pts, pst01, pst2, otp, r = state
            qsl = slice(qlo, qlo + qw)
            m0 = bp.tile([128, 512], BF16, tag="m0", bufs=2)
            nc.gpsimd.tensor_mul(m0[:, 0:qw], pst01[:, 0:qw], vsT[:, kv, 0, qsl])
            m1 = bp.tile([128, 512], BF16, tag="m1", bufs=2)
            nc.vector.tensor_mul(m1[:, 0:qw], pst01[:, 512:512 + qw], vsT[:, kv, 1, qsl])
            m2 = bp.tile([128, 512], BF16, tag="m2", bufs=2)
            nc.gpsimd.tensor_mul(m2[:, 0:qw], pst2[:, 0:qw], vsT[:, kv, 2, qsl])
            acc = bp.tile([128, 512], F32, tag="acc", bufs=2)
            nc.vector.tensor_add(acc[:, 0:qw], m0[:, 0:qw], m1[:, 0:qw])
            nc.vector.tensor_add(acc[:, 0:qw], acc[:, 0:qw], m2[:, 0:qw])
            nc.vector.tensor_add(acc[:, 0:qw], acc[:, 0:qw], otp[:, 0:qw])
            nc.vector.tensor_mul(ot[:, h, qsl], acc[:, 0:qw], r[:, 0:qw])

        def attention_scores(h, qh, prev):
            """Suffix products/colsums/exps + causal scores + exps; the
            previous block's PV/denominator groups are emitted between score
            groups so the PE has work while the Act engine computes exps."""
            kv = h // (NH // NKV)
            qlo = qh * 512
            qw = qwidth(qh)
            nki = qh * 4 + 4
            # suffix q*k elementwise products over the full 512 (qt tail is
            # zeroed when qhi < 1024, keeping the colsums finite)
            tmps = []
            for j in range(LCK):
                tmp = bp.tile([128, 512], BF16, tag=f"tmp{j}", bufs=2)
                nc.vector.tensor_mul(tmp, qt[:, h, qlo:qlo + 512],
                                     ksT[:, kv, j, qlo:qlo + 512])
                tmps.append(tmp)
            pts = []  # pt tiles [128, 1024] bf16, one per ki pair
            for g in range(nki // 2):
                stg = pp.tile([128, 1024], F32, tag="stg", bufs=2)
                for i in range(2):
                    ki = g * 2 + i
                    dd = ki * 128 - qlo
                    lo = max(0, dd)
                    if lo >= qw:
                        continue
                    c0 = i * 512 + lo
                    nc.tensor.matmul(stg[:, c0:i * 512 + qw],
                                     kt[:, kv, ki * 128:ki * 128 + 128],
                                     qt[:, h, qlo + lo:qlo + qw],
                                     start=True, stop=(dd < 0))
                    if dd >= 0:  # diagonal tile: add -1e30 where q < k
                        nc.tensor.matmul(stg[:, c0:c0 + 128],
                                         negid, ubig[:, 512:640],
                                         start=False, stop=True)
                pt = bp.tile([128, 1024], BF16, tag="pt", bufs=6)
                nc.scalar.activation(out=pt, in_=stg, func=Exp, scale=float(SCALE))
                pts.append(pt)
                if g == 0 and prev is not None:
                    out_pv(prev)
                if g == 1 and prev is not None:
                    out_sm(prev)
            # suffix colsums (PE) then exp
            sfg = pp.tile([128, 1024], F32, tag="stg", bufs=2)
            nc.tensor.matmul(sfg[:, 0:512], ones, tmps[0], start=True, stop=True)
            nc.tensor.matmul(sfg[:, 512:1024], ones, tmps[1], start=True, stop=True)
            sfg2 = pp.tile([128, 1024], F32, tag="stg", bufs=2)
            nc.tensor.matmul(sfg2[:, 0:512], ones, tmps[2], start=True, stop=True)
            pst01 = bp.tile([128, 1024], BF16, tag="pst01", bufs=2)
            nc.scalar.activation(out=pst01, in_=sfg, func=Exp, scale=float(SCALE))
            pst2 = bp.tile([128, 512], BF16, tag="pst2", bufs=2)
            nc.scalar.activation(out=pst2, in_=sfg2[:, 0:512], func=Exp, scale=float(SCALE))
            if prev is not None:
                out_combine(prev)
            return [h, qh, kv, qlo, qw, nki, pts, pst01, pst2]

        # ---------------- phase A (+ interleaved attention) --------------------
        with ExitStack() as actx:
            pa = actx.enter_context(tc.tile_pool(name="pa", bufs=1))
            wp = actx.enter_context(tc.tile_pool(name="wp", bufs=2))
            rt = actx.enter_context(tc.tile_pool(name="rt", bufs=2))

            def dma_w(w, dc):
                for q4 in range(4):
                    nc.sync.dma_start(out=w[:, q4 * 8:(q4 + 1) * 8, :],
                                      in_=w1_d[dc, :, q4 * 8:(q4 + 1) * 8, :])

            w0 = wp.tile([128, FCH, 128], BF16, tag="w")
            dma_w(w0, 0)
            hT = pa.tile([128, FCH, S], BF16, tag="hT")
            for fc in range(FCH):
                nc.sync.dma_start(out=hT[:, fc, :], in_=hT_d[fc])
            nc.sync.dma_start(out=cosT, in_=cos_d)
            nc.sync.dma_start(out=sinT, in_=sin_d)

            def rope(ps, dest, sl, qw):
                tcos = rt.tile([128, 512], F32, tag="tcos")
                nc.vector.tensor_mul(tcos[:, 0:qw], ps[:, 0:qw], cosT[:, sl])
                rot = rt.tile([128, 512], F32, tag="rot")
                nc.scalar.copy(rot[0:64, 0:qw], ps[64:128, 0:qw])
                nc.scalar.copy(rot[64:128, 0:qw], ps[0:64, 0:qw])
                tsin = rt.tile([128, 512], F32, tag="tsin")
                nc.vector.tensor_mul(tsin[:, 0:qw], rot[:, 0:qw], sinT[:, sl])
                nc.vector.tensor_add(dest, tcos[:, 0:qw], tsin[:, 0:qw])

            # dc roles: 0,1 = K kv0/kv1; 2,3 = V kv0/kv1; 4..11 = Q h0..h7
            prev = None
            for dc in range(DC):
                if dc == 0:
                    w = w0
                else:
                    w = wp.tile([128, FCH, 128], BF16, tag="w")
                    dma_w(w, dc)
                if dc == 2:
                    # suffix K/V loads deferred so they don't compete with hT
                    # for startup DMA bandwidth (first needed at dc=4)
                    for kv2 in range(NKV):
                        for j in range(LCK):
                            nc.sync.dma_start(out=ksT[:, kv2, j, :], in_=ks_d[kv2, j])
                            nc.sync.dma_start(out=vsT[:, kv2, j, :], in_=vs_d[kv2, j])
                is_q = dc >= 2 * NKV
                for sh in range(2):
                    qw = qwidth(sh) if is_q else 512
                    sl = slice(sh * 512, sh * 512 + qw)
                    ps = pp.tile([128, 512], F32, tag="ps", bufs=2)
                    for fc in range(FCH):
                        nc.tensor.matmul(ps[:, 0:qw], w[:, fc, :], hT[:, fc, sl],
                                         start=(fc == 0), stop=(fc == FCH - 1))
                    if not is_q:
                        if dc < NKV:
                            rope(ps, kt[:, dc, sl], sl, 512)
                        else:
                            kv = dc - NKV
                            vstage = rt.tile([128, 512], F32, tag="vstage")
                            nc.vector.tensor_copy(out=vstage, in_=ps)
                            tp = pp.tile([128, 512], F32, tag="ps", bufs=2)
                            for t4 in range(4):
                                nc.tensor.transpose(tp[:, t4 * 128:(t4 + 1) * 128],
                                                    vstage[:, t4 * 128:(t4 + 1) * 128],
                                                    ident)
                            nc.vector.tensor_copy(out=vn[:, kv, sh * 4:(sh + 1) * 4, :], in_=tp)
                    else:
                        h = dc - 2 * NKV
                        rope(ps, qt[:, h, sl], sl, qw)
                        prev = attention_scores(h, sh, prev)
            # flush the last attention block
            out_pv(prev)
            out_sm(prev)
            out_combine(prev)

        # ---------------- phase C: output projection -----------------------
        with ExitStack() as cctx:
            wp2 = cctx.enter_context(tc.tile_pool(name="wp2", bufs=1))
            for ncol in range(8):
                nsl = slice(ncol * 512, ncol * 512 + 512)
                wo_t = wp2.tile([128, NH, 512], BF16, tag="wo", bufs=3)
                for h in range(NH):
                    nc.sync.dma_start(out=wo_t[:, h, :], in_=wo_d[h, :, nsl])
                for scp in range(4):
                    fo = pp.tile([128, 1024], F32, tag="stg", bufs=2)
                    for half in range(2):
                        sc = scp * 2 + half
                        for h in range(NH):
                            nc.tensor.matmul(fo[:, half * 512:half * 512 + 512],
                                             ot[:, h, sc * 128:(sc + 1) * 128],
                                             wo_t[:, h, :], start=(h == 0), stop=(h == NH - 1))
                        fo_sb = wp2.tile([128, 512], F32, tag="fosb", bufs=4)
                        nc.vector.tensor_copy(out=fo_sb, in_=fo[:, half * 512:half * 512 + 512])
                        nc.sync.dma_start(out=out_d[sc * 128:(sc + 1) * 128, nsl], in_=fo_sb)
    nc.compile()
    return nc


_NC = {}
_LAST_QHI = 1024


def _get_nc(qhi=None):
    global _LAST_QHI
    if qhi is None:
        qhi = _LAST_QHI
    _LAST_QHI = qhi
    if qhi not in _NC:
        _NC[qhi] = _build(qhi)
    return _NC[qhi]


def kernel(hidden_states, k_suffix, v_suffix, Wq, Wk, Wv, Wo, valid_seq_len):
    B = hidden_states.shape[0]
    valid = int(np.asarray(valid_seq_len))
    qhi = valid if 512 < valid < 1024 else 1024

    # rope tables, transposed to [d, s], sin sign-folded for rotate_half
    inv_freq = 1.0 / (10000.0 ** (np.arange(0, D, 2, dtype=np.float32) / D))
    pos = np.arange(S, dtype=np.float32)
    freqs = pos[:, None] * inv_freq[None, :]
    emb = np.concatenate([freqs, freqs], axis=-1)          # [S, D]
    cosT = np.cos(emb).T.astype(np.float32).copy()         # [D, S]
    sinT = np.sin(emb).T.astype(np.float32).copy()
    sgn = np.where(np.arange(D) < D // 2, -1.0, 1.0).astype(np.float32)
    sinT = sinT * sgn[:, None]

    in_maps = []
    for core in range(8):
        b = core // 4
        hg = core % 4
        qsl = slice(hg * NH * D, (hg + 1) * NH * D)
        kvsl = slice(hg * NKV * D, (hg + 1) * NKV * D)

        hT = np.ascontiguousarray(hidden_states[b].T).reshape(FCH, 128, S)
        # dc order: K(2), V(2), Q(8)
        w1 = np.concatenate([Wk[:, kvsl], Wv[:, kvsl], Wq[:, qsl]], axis=1)  # [4096, 1536]
        w1 = w1.reshape(FCH, 128, DC, 128).transpose(2, 1, 0, 3)             # [DC, 128p, FCH, 128m]
        ks = k_suffix[b, hg * NKV:(hg + 1) * NKV].transpose(0, 1, 3, 2)      # [NKV, LCK, 128d, S]
        vs = v_suffix[b, hg * NKV:(hg + 1) * NKV].transpose(0, 1, 3, 2)
        wo = Wo[hg * NH * D:(hg + 1) * NH * D].reshape(NH, 128, HIDDEN)

        in_maps.append({
            "hT": hT.astype(ml_dtypes.bfloat16),
            "w1": np.ascontiguousarray(w1).astype(ml_dtypes.bfloat16),
            "cosT": cosT.astype(ml_dtypes.bfloat16),
            "sinT": sinT.astype(ml_dtypes.bfloat16),
            "ksT": np.ascontiguousarray(ks).astype(ml_dtypes.bfloat16),
            "vsT": np.ascontiguousarray(vs).astype(ml_dtypes.bfloat16),
            "wo": np.ascontiguousarray(wo).astype(ml_dtypes.bfloat16),
        })

    global _LAST_IN_MAPS
    _LAST_IN_MAPS = in_maps
    nc = _get_nc(qhi)
    res = run_bass_kernel_spmd(nc, in_maps, core_ids=list(range(8)))

    out = np.zeros((B, S, HIDDEN), dtype=np.float32)
    for core in range(8):
        out[core // 4] += res.results[core]["out"]
    out[:, valid:, :] = 0.0
    return out


if __name__ == "__main__":
    rng = np.random.default_rng(0)
    h = rng.standard_normal((2, S, HIDDEN)).astype(np.float32)
    ks = rng.standard_normal((2, 8, LCK, S, D)).astype(np.float32)
    vs = rng.standard_normal((2, 8, LCK, S, D)).astype(np.float32)
    wq = (rng.standard_normal((HIDDEN, HIDDEN)) * 0.02).astype(np.float32)
    wk = (rng.standard_normal((HIDDEN, 1024)) * 0.02).astype(np.float32)
    wv = (rng.standard_normal((HIDDEN, 1024)) * 0.02).astype(np.float32)
    wo = (rng.standard_normal((HIDDEN, HIDDEN)) * 0.02).astype(np.float32)
    o = kernel(hidden_states=h, k_suffix=ks, v_suffix=vs, Wq=wq, Wk=wk, Wv=wv, Wo=wo,
               valid_seq_len=960)
    print(o.shape, o.dtype, np.abs(o).max())


# revision 24
# speedup vs baseline: 1.0460x; 1.0460x over previous
"""Trainium2 Bass kernel for LlamaFlashAttentionMasked (EAGLE3 suffix-block attention).

Sharding: 8 cores = batch(2) x head-group(4). Each core handles 1 batch and
8 q-heads / 2 kv-heads. Per-core partial outputs (after Wo on the core's head
slice) are summed across the 4 head-groups on the host.

All-bf16 matmuls; causal mask folded into the score matmul group on the PE
(neg-identity stationary x shifted-triangle moving); 1024-wide PSUM exp
groups; suffix denominators folded into the ones-matmul; suffix q*k products
on DVE bf16; K/V projected first so each head's attention interleaves with
the remaining Q projections; each attention block's PV/denominator groups are
emitted between the NEXT block's score groups so the PE never waits on the
Act engine's exp latency; queries >= valid_seq_len are skipped.
"""
import sys
sys.path.insert(0, "/opt/trn_rl_repo")

from contextlib import ExitStack, contextmanager

import numpy as np
import ml_dtypes

import concourse.bacc as bacc
import concourse.tile as tile
import concourse.mybir as mybir
from concourse.bass_utils import run_bass_kernel_spmd
from concourse.masks import make_identity

F32 = mybir.dt.float32
BF16 = mybir.dt.bfloat16
Exp = mybir.ActivationFunctionType.Exp

HIDDEN = 4096
S = 1024
NH = 8        # q heads per core
NKV = 2       # kv heads per core
D = 128
LCK = 3
FCH = HIDDEN // 128   # 32 f-chunks
DC = NKV + NKV + NH   # 12 projection output chunks: 2 K, 2 V, 8 Q
SCALE = 1.0 / np.sqrt(D)
NEG = -1e30


def tc_ctx(nc):
    @contextmanager
    def _cm():
        with tile.TileContext(nc) as tc:
            with ExitStack() as ctx:
                yield tc, ctx
    return _cm()


def _build(qhi=1024):
    """qhi: number of live query rows (valid_seq_len clamped to (512, 1024])."""
    w1 = qhi - 512  # width of the second query half

    nc = bacc.Bacc("TRN2", target_bir_lowering=False, debug=False, num_devices=8)

    hT_d = nc.dram_tensor("hT", [FCH, 128, S], BF16, kind="ExternalInput").ap()
    w1_d = nc.dram_tensor("w1", [DC, 128, FCH, 128], BF16, kind="ExternalInput").ap()
    cos_d = nc.dram_tensor("cosT", [128, S], BF16, kind="ExternalInput").ap()
    sin_d = nc.dram_tensor("sinT", [128, S], BF16, kind="ExternalInput").ap()
    ks_d = nc.dram_tensor("ksT", [NKV, LCK, 128, S], BF16, kind="ExternalInput").ap()
    vs_d = nc.dram_tensor("vsT", [NKV, LCK, 128, S], BF16, kind="ExternalInput").ap()
    wo_d = nc.dram_tensor("wo", [NH, 128, HIDDEN], BF16, kind="ExternalInput").ap()
    out_d = nc.dram_tensor("out", [S, HIDDEN], F32, kind="ExternalOutput").ap()

    with tc_ctx(nc) as (tc, ctx):
        pers = ctx.enter_context(tc.tile_pool(name="pers", bufs=1))
        qt = pers.tile([128, NH, S], BF16, tag="qt")        # roped Q^T per head
        kt = pers.tile([128, NKV, S], BF16, tag="kt")       # roped K^T per kv head
        vn = pers.tile([128, NKV, 8, D], BF16, tag="vn")    # V natural [s-part, kv, s-chunk, d]
        ot = pers.tile([128, NH, S], BF16, tag="ot")        # normalized attn out (lhsT for Wo)
        ksT = pers.tile([128, NKV, LCK, S], BF16, tag="ksT")
        vsT = pers.tile([128, NKV, LCK, S], BF16, tag="vsT")
        cosT = pers.tile([128, S], BF16, tag="cos")
        sinT = pers.tile([128, S], BF16, tag="sin")
        ones = pers.tile([128, 128], BF16, tag="ones")
        nc.vector.memset(ones, 1.0)
        oneo = pers.tile([128, 128], BF16, tag="oneo")      # 1/128 for replicated colsums
        nc.vector.memset(oneo, 1.0 / 128.0)
        ident = pers.tile([128, 128], F32, tag="ident")
        make_identity(nc, ident)
        negid = pers.tile([128, 128], BF16, tag="negid")    # -1e30 * I
        nc.vector.tensor_scalar_mul(negid, ident, NEG)
        # ubig[p, j] = 1 if j < p + 512 else 0  (shifted causal triangle)
        ubig = pers.tile([128, 640], BF16, tag="ubig")
        nc.gpsimd.memset(ubig, 1.0)
        nc.gpsimd.affine_select(
            out=ubig, in_=ubig,
            compare_op=mybir.AluOpType.is_ge,
            fill=0.0, base=511,
            pattern=[[-1, 640]], channel_multiplier=1,
        )
        if qhi < 1024:
            # dropped query rows: zero so suffix products and phase C read
            # well-defined values
            nc.vector.memset(qt[:, :, qhi:1024], 0.0)
            nc.vector.memset(ot[:, :, qhi:1024], 0.0)

        # B-phase temp pool (opened before the A pool so closing A frees space
        # for the C pool)
        bp = ctx.enter_context(tc.tile_pool(name="bp", bufs=1))
        # single shared PSUM pool: ps(2) + stg(4) + sm(1) + otp(1) = 8 banks
        pp = ctx.enter_context(tc.tile_pool(name="pp", bufs=1, space="PSUM"))
        # initialize both stg buffers: trapezoidal score tiles leave their
        # fully-masked column ranges unwritten, and the group exp reads them
        for _ in range(2):
            stg0 = pp.tile([128, 1024], F32, tag="stg", bufs=2)
            nc.vector.memset(stg0, 0.0)

        def qwidth(qh):
            return 512 if qh == 0 else w1

        def out_pv(state):
            h, qh, kv, qlo, qw, nki, pts, pst01, pst2 = state
            otp = pp.tile([128, 512], F32, tag="otp", bufs=1)
            for ki in range(nki):
                dd = max(0, ki * 128 - qlo)
                nc.tensor.matmul(otp[:, dd:qw], vn[:, kv, ki, :],
                                 pts[ki // 2][:, (ki % 2) * 512 + dd:(ki % 2) * 512 + qw],
                                 start=(ki == 0), stop=(ki == nki - 1))
            state.append(otp)

        def out_sm(state):
            h, qh, kv, qlo, qw, nki, pts, pst01, pst2, otp = state
            # denominator: replicated suffix exps scaled by 1/128 first, then
            # colsums of the exp'd causal tiles
            sm = pp.tile([128, 512], F32, tag="sm", bufs=1)
            nc.tensor.matmul(sm, oneo, pst01[:, 0:512], start=True, stop=False)
            nc.tensor.matmul(sm, oneo, pst01[:, 512:1024], start=False, stop=False)
            nc.tensor.matmul(sm, oneo, pst2, start=False, stop=False)
            for ki in range(nki):
                dd = max(0, ki * 128 - qlo)
                nc.tensor.matmul(sm[:, dd:qw], ones,
                                 pts[ki // 2][:, (ki % 2) * 512 + dd:(ki % 2) * 512 + qw],
                                 start=False, stop=(ki == nki - 1))
            r = bp.tile([128, 512], F32, tag="r", bufs=2)
            nc.vector.reciprocal_approx_fast(out=r, in_=sm)
            state.append(r)

        def out_combine(state):
            h, qh, kv, qlo, qw, nki, pts, pst01, pst2, otp, r = state
            qsl = slice(qlo, qlo + qw)
            m0 = bp.tile([128, 512], BF16, tag="m0", bufs=2)
            nc.gpsimd.tensor_mul(m0[:, 0:qw], pst01[:, 0:qw], vsT[:, kv, 0, qsl])
            m1 = bp.tile([128, 512], BF16, tag="m1", bufs=2)
            nc.vector.tensor_mul(m1[:, 0:qw], pst01[:, 512:512 + qw], vsT[:, kv, 1, qsl])
            m2 = bp.tile([128, 512], BF16, tag="m2", bufs=2)
            nc.gpsimd.tensor_mul(m2[:, 0:qw], pst2[:, 0:qw], vsT[:, kv, 2, qsl])
            # read otp first so its psum bank frees as early as possible
            acc = bp.tile([128, 512], F32, tag="acc", bufs=2)
            nc.vector.tensor_add(acc[:, 0:qw], m1[:, 0:qw], otp[:, 0:qw])
            nc.vector.tensor_add(acc[:, 0:qw], acc[:, 0:qw], m0[:, 0:qw])
            nc.vector.tensor_add(acc[:, 0:qw], acc[:, 0:qw], m2[:, 0:qw])
            nc.vector.tensor_mul(ot[:, h, qsl], acc[:, 0:qw], r[:, 0:qw])

        def attention_scores(h, qh):
            """Suffix products/colsums/exps + causal scores + exps; the
            PV/denominator groups are emitted one proj chunk later so the PE
            never waits on the Act engine's exp latency."""
            kv = h // (NH // NKV)
            qlo = qh * 512
            qw = qwidth(qh)
            nki = qh * 4 + 4
            # suffix q*k elementwise products over the full 512 (qt tail is
            # zeroed when qhi < 1024, keeping the colsums finite)
            tmps = []
            for j in range(LCK):
                tmp = bp.tile([128, 512], BF16, tag=f"tmp{j}", bufs=2)
                nc.vector.tensor_mul(tmp, qt[:, h, qlo:qlo + 512],
                                     ksT[:, kv, j, qlo:qlo + 512])
                tmps.append(tmp)
            pts = []  # pt tiles [128, 1024] bf16, one per ki pair
            for g in range(nki // 2):
                stg = pp.tile([128, 1024], F32, tag="stg", bufs=2)
                for i in range(2):
                    ki = g * 2 + i
                    dd = ki * 128 - qlo
                    lo = max(0, dd)
                    if lo >= qw:
                        continue
                    c0 = i * 512 + lo
                    nc.tensor.matmul(stg[:, c0:i * 512 + qw],
                                     kt[:, kv, ki * 128:ki * 128 + 128],
                                     qt[:, h, qlo + lo:qlo + qw],
                                     start=True, stop=(dd < 0))
                    if dd >= 0:  # diagonal tile: add -1e30 where q < k
                        nc.tensor.matmul(stg[:, c0:c0 + 128],
                                         negid, ubig[:, 512:640],
                                         start=False, stop=True)
                pt = bp.tile([128, 1024], BF16, tag="pt", bufs=6)
                nc.scalar.activation(out=pt, in_=stg, func=Exp, scale=float(SCALE))
                pts.append(pt)
            # suffix colsums (PE) then exp
            sfg = pp.tile([128, 1024], F32, tag="stg", bufs=2)
            nc.tensor.matmul(sfg[:, 0:512], ones, tmps[0], start=True, stop=True)
            nc.tensor.matmul(sfg[:, 512:1024], ones, tmps[1], start=True, stop=True)
            sfg2 = pp.tile([128, 1024], F32, tag="stg", bufs=2)
            nc.tensor.matmul(sfg2[:, 0:512], ones, tmps[2], start=True, stop=True)
            pst01 = bp.tile([128, 1024], BF16, tag="pst01", bufs=2)
            nc.scalar.activation(out=pst01, in_=sfg, func=Exp, scale=float(SCALE))
            pst2 = bp.tile([128, 512], BF16, tag="pst2", bufs=2)
            nc.scalar.activation(out=pst2, in_=sfg2[:, 0:512], func=Exp, scale=float(SCALE))
            return [h, qh, kv, qlo, qw, nki, pts, pst01, pst2]

        def attention_out(state):
            out_pv(state)
            out_sm(state)
            out_combine(state)

        # ---------------- phase A (+ interleaved attention) --------------------
        with ExitStack() as actx:
            pa = actx.enter_context(tc.tile_pool(name="pa", bufs=1))
            wp = actx.enter_context(tc.tile_pool(name="wp", bufs=2))
            rt = actx.enter_context(tc.tile_pool(name="rt", bufs=2))

            def dma_w(w, dc):
                for q4 in range(4):
                    nc.sync.dma_start(out=w[:, q4 * 8:(q4 + 1) * 8, :],
                                      in_=w1_d[dc, :, q4 * 8:(q4 + 1) * 8, :])

            w0 = wp.tile([128, FCH, 128], BF16, tag="w")
            dma_w(w0, 0)
            hT = pa.tile([128, FCH, S], BF16, tag="hT")
            for fc in range(FCH):
                nc.sync.dma_start(out=hT[:, fc, :], in_=hT_d[fc])
            nc.sync.dma_start(out=cosT, in_=cos_d)
            nc.sync.dma_start(out=sinT, in_=sin_d)

            def rope(ps, dest, sl, qw):
                tcos = rt.tile([128, 512], F32, tag="tcos")
                nc.vector.tensor_mul(tcos[:, 0:qw], ps[:, 0:qw], cosT[:, sl])
                rot = rt.tile([128, 512], F32, tag="rot")
                nc.scalar.copy(rot[0:64, 0:qw], ps[64:128, 0:qw])
                nc.scalar.copy(rot[64:128, 0:qw], ps[0:64, 0:qw])
                tsin = rt.tile([128, 512], F32, tag="tsin")
                nc.vector.tensor_mul(tsin[:, 0:qw], rot[:, 0:qw], sinT[:, sl])
                nc.vector.tensor_add(dest, tcos[:, 0:qw], tsin[:, 0:qw])

            # dc roles: 0,1 = K kv0/kv1; 2,3 = V kv0/kv1; 4..11 = Q h0..h7
            pending = []
            for dc in range(DC):
                if dc == 0:
                    w = w0
                else:
                    w = wp.tile([128, FCH, 128], BF16, tag="w")
                    dma_w(w, dc)
                if dc == 2:
                    # suffix K/V loads deferred so they don't compete with hT
                    # for startup DMA bandwidth (first needed at dc=4)
                    for kv2 in range(NKV):
                        for j in range(LCK):
                            nc.sync.dma_start(out=ksT[:, kv2, j, :], in_=ks_d[kv2, j])
                            nc.sync.dma_start(out=vsT[:, kv2, j, :], in_=vs_d[kv2, j])
                is_q = dc >= 2 * NKV
                for sh in range(2):
                    qw = qwidth(sh) if is_q else 512
                    sl = slice(sh * 512, sh * 512 + qw)
                    ps = pp.tile([128, 512], F32, tag="ps", bufs=2)
                    for fc in range(FCH):
                        nc.tensor.matmul(ps[:, 0:qw], w[:, fc, :], hT[:, fc, sl],
                                         start=(fc == 0), stop=(fc == FCH - 1))
                    if pending:
                        attention_out(pending.pop(0))
                    if not is_q:
                        if dc < NKV:
                            rope(ps, kt[:, dc, sl], sl, 512)
                        else:
                            kv = dc - NKV
                            vstage = rt.tile([128, 512], F32, tag="vstage")
                            nc.vector.tensor_copy(out=vstage, in_=ps)
                            tp = pp.tile([128, 512], F32, tag="ps", bufs=2)
                            for t4 in range(4):
                                nc.tensor.transpose(tp[:, t4 * 128:(t4 + 1) * 128],
                                                    vstage[:, t4 * 128:(t4 + 1) * 128],
                                                    ident)
                            nc.vector.tensor_copy(out=vn[:, kv, sh * 4:(sh + 1) * 4, :], in_=tp)
                    else:
                        h = dc - 2 * NKV
                        rope(ps, qt[:, h, sl], sl, qw)
                        pending.append(attention_scores(h, sh))
            while pending:
                attention_out(pending.pop(0))

        # ---------------- phase C: output projection -----------------------
        with ExitStack() as cctx:
            wp2 = cctx.enter_context(tc.tile_pool(name="wp2", bufs=1))
            for ncol in range(8):
                nsl = slice(ncol * 512, ncol * 512 + 512)
                wo_t = wp2.tile([128, NH, 512], BF16, tag="wo", bufs=3)
                for h in range(NH):
                    nc.sync.dma_start(out=wo_t[:, h, :], in_=wo_d[h, :, nsl])
                for scp in range(4):
                    fo = pp.tile([128, 1024], F32, tag="stg", bufs=2)
                    for half in range(2):
                        sc = scp * 2 + half
                        for h in range(NH):
                            nc.tensor.matmul(fo[:, half * 512:half * 512 + 512],
                                             ot[:, h, sc * 128:(sc + 1) * 128],
                                             wo_t[:, h, :], start=(h == 0), stop=(h == NH - 1))
                        fo_sb = wp2.tile([128, 512], F32, tag="fosb", bufs=4)
                        nc.vector.tensor_copy(out=fo_sb, in_=fo[:, half * 512:half * 512 + 512])
                        nc.sync.dma_start(out=out_d[sc * 128:(sc + 1) * 128, nsl], in_=fo_sb)
    nc.compile()
    return nc


_NC = {}
_LAST_QHI = 1024


def _get_nc(qhi=None):
    global _LAST_QHI
    if qhi is None:
        qhi = _LAST_QHI
    _LAST_QHI = qhi
    if qhi not in _NC:
        _NC[qhi] = _build(qhi)
    return _NC[qhi]


def kernel(hidden_states, k_suffix, v_suffix, Wq, Wk, Wv, Wo, valid_seq_len):
    B = hidden_states.shape[0]
    valid = int(np.asarray(valid_seq_len))
    qhi = valid if 512 < valid < 1024 else 1024

    # rope tables, transposed to [d, s], sin sign-folded for rotate_half
    inv_freq = 1.0 / (10000.0 ** (np.arange(0, D, 2, dtype=np.float32) / D))
    pos = np.arange(S, dtype=np.float32)
    freqs = pos[:, None] * inv_freq[None, :]
    emb = np.concatenate([freqs, freqs], axis=-1)          # [S, D]
    cosT = np.cos(emb).T.astype(np.float32).copy()         # [D, S]
    sinT = np.sin(emb).T.astype(np.float32).copy()
    sgn = np.where(np.arange(D) < D // 2, -1.0, 1.0).astype(np.float32)
    sinT = sinT * sgn[:, None]

    in_maps = []
    for core in range(8):
        b = core // 4
        hg = core % 4
        qsl = slice(hg * NH * D, (hg + 1) * NH * D)
        kvsl = slice(hg * NKV * D, (hg + 1) * NKV * D)

        hT = np.ascontiguousarray(hidden_states[b].T).reshape(FCH, 128, S)
        # dc order: K(2), V(2), Q(8)
        w1 = np.concatenate([Wk[:, kvsl], Wv[:, kvsl], Wq[:, qsl]], axis=1)  # [4096, 1536]
        w1 = w1.reshape(FCH, 128, DC, 128).transpose(2, 1, 0, 3)             # [DC, 128p, FCH, 128m]
        ks = k_suffix[b, hg * NKV:(hg + 1) * NKV].transpose(0, 1, 3, 2)      # [NKV, LCK, 128d, S]
        vs = v_suffix[b, hg * NKV:(hg + 1) * NKV].transpose(0, 1, 3, 2)
        wo = Wo[hg * NH * D:(hg + 1) * NH * D].reshape(NH, 128, HIDDEN)

        in_maps.append({
            "hT": hT.astype(ml_dtypes.bfloat16),
            "w1": np.ascontiguousarray(w1).astype(ml_dtypes.bfloat16),
            "cosT": cosT.astype(ml_dtypes.bfloat16),
            "sinT": sinT.astype(ml_dtypes.bfloat16),
            "ksT": np.ascontiguousarray(ks).astype(ml_dtypes.bfloat16),
            "vsT": np.ascontiguousarray(vs).astype(ml_dtypes.bfloat16),
            "wo": np.ascontiguousarray(wo).astype(ml_dtypes.bfloat16),
        })

    global _LAST_IN_MAPS
    _LAST_IN_MAPS = in_maps
    nc = _get_nc(qhi)
    res = run_bass_kernel_spmd(nc, in_maps, core_ids=list(range(8)))

    out = np.zeros((B, S, HIDDEN), dtype=np.float32)
    for core in range(8):
        out[core // 4] += res.results[core]["out"]
    out[:, valid:, :] = 0.0
    return out


if __name__ == "__main__":
    rng = np.random.default_rng(0)
    h = rng.standard_normal((2, S, HIDDEN)).astype(np.float32)
    ks = rng.standard_normal((2, 8, LCK, S, D)).astype(np.float32)
    vs = rng.standard_normal((2, 8, LCK, S, D)).astype(np.float32)
    wq = (rng.standard_normal((HIDDEN, HIDDEN)) * 0.02).astype(np.float32)
    wk = (rng.standard_normal((HIDDEN, 1024)) * 0.02).astype(np.float32)
    wv = (rng.standard_normal((HIDDEN, 1024)) * 0.02).astype(np.float32)
    wo = (rng.standard_normal((HIDDEN, HIDDEN)) * 0.02).astype(np.float32)
    o = kernel(hidden_states=h, k_suffix=ks, v_suffix=vs, Wq=wq, Wk=wk, Wv=wv, Wo=wo,
               valid_seq_len=960)
    print(o.shape, o.dtype, np.abs(o).max())


# revision 27
# speedup vs baseline: 1.0493x; 1.0031x over previous
"""Trainium2 Bass kernel for LlamaFlashAttentionMasked (EAGLE3 suffix-block attention).

Sharding: 8 cores = batch(2) x head-group(4). Each core handles 1 batch and
8 q-heads / 2 kv-heads. Per-core partial outputs (after Wo on the core's head
slice) are summed across the 4 head-groups on the host.

All-bf16 matmuls; causal mask folded into the score matmul group on the PE
(neg-identity stationary x shifted-triangle moving); 1024-wide PSUM exp
groups; suffix denominators folded into the ones-matmul; suffix q*k products
on DVE bf16; K/V projected first so each head's attention interleaves with
the remaining Q projections; each attention block's PV/denominator groups are
emitted between the NEXT block's score groups so the PE never waits on the
Act engine's exp latency; queries >= valid_seq_len are skipped.
"""
import sys
sys.path.insert(0, "/opt/trn_rl_repo")

from contextlib import ExitStack, contextmanager

import numpy as np
import ml_dtypes

import concourse.bacc as bacc
import concourse.tile as tile
import concourse.mybir as mybir
from concourse.bass_utils import run_bass_kernel_spmd
from concourse.masks import make_identity

F32 = mybir.dt.float32
BF16 = mybir.dt.bfloat16
Exp = mybir.ActivationFunctionType.Exp

HIDDEN = 4096
S = 1024
NH = 8        # q heads per core
NKV = 2       # kv heads per core
D = 128
LCK = 3
FCH = HIDDEN // 128   # 32 f-chunks
DC = NKV + NKV + NH   # 12 projection output chunks: 2 K, 2 V, 8 Q
SCALE = 1.0 / np.sqrt(D)
NEG = -1e30


def tc_ctx(nc):
    @contextmanager
    def _cm():
        with tile.TileContext(nc) as tc:
            with ExitStack() as ctx:
                yield tc, ctx
    return _cm()


def _build(qhi=1024):
    """qhi: number of live query rows (valid_seq_len clamped to (512, 1024])."""
    w1 = qhi - 512  # width of the second query half

    nc = bacc.Bacc("TRN2", target_bir_lowering=False, debug=False, num_devices=8)

    hT_d = nc.dram_tensor("hT", [FCH, 128, S], BF16, kind="ExternalInput").ap()
    w1_d = nc.dram_tensor("w1", [DC, 128, FCH, 128], BF16, kind="ExternalInput").ap()
    cos_d = nc.dram_tensor("cosT", [128, S], BF16, kind="ExternalInput").ap()
    sin_d = nc.dram_tensor("sinT", [128, S], BF16, kind="ExternalInput").ap()
    ks_d = nc.dram_tensor("ksT", [NKV, LCK, 128, S], BF16, kind="ExternalInput").ap()
    vs_d = nc.dram_tensor("vsT", [NKV, LCK, 128, S], BF16, kind="ExternalInput").ap()
    wo_d = nc.dram_tensor("wo", [NH, 128, HIDDEN], BF16, kind="ExternalInput").ap()
    out_d = nc.dram_tensor("out", [S, HIDDEN], F32, kind="ExternalOutput").ap()

    with tc_ctx(nc) as (tc, ctx):
        pers = ctx.enter_context(tc.tile_pool(name="pers", bufs=1))
        qt = pers.tile([128, NH, S], BF16, tag="qt")        # roped Q^T per head
        kt = pers.tile([128, NKV, S], BF16, tag="kt")       # roped K^T per kv head
        vn = pers.tile([128, NKV, 8, D], BF16, tag="vn")    # V natural [s-part, kv, s-chunk, d]
        ot = pers.tile([128, NH, S], BF16, tag="ot")        # normalized attn out (lhsT for Wo)
        ksT = pers.tile([128, NKV, LCK, S], BF16, tag="ksT")
        vsT = pers.tile([128, NKV, LCK, S], BF16, tag="vsT")
        cosT = pers.tile([128, S], BF16, tag="cos")
        sinT = pers.tile([128, S], BF16, tag="sin")
        ones = pers.tile([128, 128], BF16, tag="ones")
        nc.vector.memset(ones, 1.0)
        oneo = pers.tile([128, 128], BF16, tag="oneo")      # 1/128 for replicated colsums
        nc.vector.memset(oneo, 1.0 / 128.0)
        ident = pers.tile([128, 128], F32, tag="ident")
        make_identity(nc, ident)
        negid = pers.tile([128, 128], BF16, tag="negid")    # -1e30 * I
        nc.vector.tensor_scalar_mul(negid, ident, NEG)
        # ubig[p, j] = 1 if j < p + 512 else 0  (shifted causal triangle)
        ubig = pers.tile([128, 640], BF16, tag="ubig")
        nc.gpsimd.memset(ubig, 1.0)
        nc.gpsimd.affine_select(
            out=ubig, in_=ubig,
            compare_op=mybir.AluOpType.is_ge,
            fill=0.0, base=511,
            pattern=[[-1, 640]], channel_multiplier=1,
        )
        if qhi < 1024:
            # dropped query rows: zero so suffix products and phase C read
            # well-defined values
            nc.vector.memset(qt[:, :, qhi:1024], 0.0)
            nc.vector.memset(ot[:, :, qhi:1024], 0.0)

        # B-phase temp pool (opened before the A pool so closing A frees space
        # for the C pool)
        bp = ctx.enter_context(tc.tile_pool(name="bp", bufs=1))
        # single shared PSUM pool: ps(2) + stg(4) + sm(1) + otp(1) = 8 banks
        pp = ctx.enter_context(tc.tile_pool(name="pp", bufs=1, space="PSUM"))
        # initialize both stg buffers: trapezoidal score tiles leave their
        # fully-masked column ranges unwritten, and the group exp reads them
        for _ in range(2):
            stg0 = pp.tile([128, 1024], F32, tag="stg", bufs=2)
            nc.vector.memset(stg0, 0.0)

        def qwidth(qh):
            return 512 if qh == 0 else w1

        def out_pv(state):
            h, qh, kv, qlo, qw, nki, pts, pst01, pst2 = state
            otp = pp.tile([128, 512], F32, tag="otp", bufs=1)
            for ki in range(nki):
                dd = max(0, ki * 128 - qlo)
                nc.tensor.matmul(otp[:, dd:qw], vn[:, kv, ki, :],
                                 pts[ki // 2][:, (ki % 2) * 512 + dd:(ki % 2) * 512 + qw],
                                 start=(ki == 0), stop=(ki == nki - 1))
            state.append(otp)

        def out_sm(state):
            h, qh, kv, qlo, qw, nki, pts, pst01, pst2, otp = state
            # denominator: replicated suffix exps scaled by 1/128 first, then
            # colsums of the exp'd causal tiles
            sm = pp.tile([128, 512], F32, tag="sm", bufs=1)
            nc.tensor.matmul(sm, oneo, pst01[:, 0:512], start=True, stop=False)
            nc.tensor.matmul(sm, oneo, pst01[:, 512:1024], start=False, stop=False)
            nc.tensor.matmul(sm, oneo, pst2, start=False, stop=False)
            for ki in range(nki):
                dd = max(0, ki * 128 - qlo)
                nc.tensor.matmul(sm[:, dd:qw], ones,
                                 pts[ki // 2][:, (ki % 2) * 512 + dd:(ki % 2) * 512 + qw],
                                 start=False, stop=(ki == nki - 1))
            r = bp.tile([128, 512], F32, tag="r", bufs=2)
            nc.vector.reciprocal_approx_fast(out=r, in_=sm)
            state.append(r)

        def out_combine(state):
            h, qh, kv, qlo, qw, nki, pts, pst01, pst2, otp, r = state
            qsl = slice(qlo, qlo + qw)
            m0 = bp.tile([128, 512], BF16, tag="m0", bufs=1)
            nc.gpsimd.tensor_mul(m0[:, 0:qw], pst01[:, 0:qw], vsT[:, kv, 0, qsl])
            m1 = bp.tile([128, 512], BF16, tag="m1", bufs=1)
            nc.vector.tensor_mul(m1[:, 0:qw], pst01[:, 512:512 + qw], vsT[:, kv, 1, qsl])
            m2 = bp.tile([128, 512], BF16, tag="m2", bufs=1)
            nc.gpsimd.tensor_mul(m2[:, 0:qw], pst2[:, 0:qw], vsT[:, kv, 2, qsl])
            # read otp first so its psum bank frees as early as possible
            acc = bp.tile([128, 512], F32, tag="acc", bufs=1)
            nc.vector.tensor_add(acc[:, 0:qw], m1[:, 0:qw], otp[:, 0:qw])
            nc.vector.tensor_add(acc[:, 0:qw], acc[:, 0:qw], m0[:, 0:qw])
            nc.vector.tensor_add(acc[:, 0:qw], acc[:, 0:qw], m2[:, 0:qw])
            nc.vector.tensor_mul(ot[:, h, qsl], acc[:, 0:qw], r[:, 0:qw])

        def attention_scores(h, qh):
            """Suffix products/colsums/exps + causal scores + exps; the
            PV/denominator groups are emitted one proj chunk later so the PE
            never waits on the Act engine's exp latency."""
            kv = h // (NH // NKV)
            qlo = qh * 512
            qw = qwidth(qh)
            nki = qh * 4 + 4
            # suffix q*k elementwise products over the full 512 (qt tail is
            # zeroed when qhi < 1024, keeping the colsums finite)
            tmps = []
            for j in range(LCK):
                tmp = bp.tile([128, 512], BF16, tag=f"tmp{j}", bufs=2)
                nc.vector.tensor_mul(tmp, qt[:, h, qlo:qlo + 512],
                                     ksT[:, kv, j, qlo:qlo + 512])
                tmps.append(tmp)
            pts = []  # pt tiles [128, 1024] bf16, one per ki pair
            for g in range(nki // 2):
                stg = pp.tile([128, 1024], F32, tag="stg", bufs=2)
                for i in range(2):
                    ki = g * 2 + i
                    dd = ki * 128 - qlo
                    lo = max(0, dd)
                    if lo >= qw:
                        continue
                    c0 = i * 512 + lo
                    nc.tensor.matmul(stg[:, c0:i * 512 + qw],
                                     kt[:, kv, ki * 128:ki * 128 + 128],
                                     qt[:, h, qlo + lo:qlo + qw],
                                     start=True, stop=(dd < 0))
                    if dd >= 0:  # diagonal tile: add -1e30 where q < k
                        nc.tensor.matmul(stg[:, c0:c0 + 128],
                                         negid, ubig[:, 512:640],
                                         start=False, stop=True)
                pt = bp.tile([128, 1024], BF16, tag="pt", bufs=6)
                nc.scalar.activation(out=pt, in_=stg, func=Exp, scale=float(SCALE))
                pts.append(pt)
            # suffix colsums (PE) then exp
            sfg = pp.tile([128, 1024], F32, tag="stg", bufs=2)
            nc.tensor.matmul(sfg[:, 0:512], ones, tmps[0], start=True, stop=True)
            nc.tensor.matmul(sfg[:, 512:1024], ones, tmps[1], start=True, stop=True)
            sfg2 = pp.tile([128, 1024], F32, tag="stg", bufs=2)
            nc.tensor.matmul(sfg2[:, 0:512], ones, tmps[2], start=True, stop=True)
            pst01 = bp.tile([128, 1024], BF16, tag="pst01", bufs=2)
            nc.scalar.activation(out=pst01, in_=sfg, func=Exp, scale=float(SCALE))
            pst2 = bp.tile([128, 512], BF16, tag="pst2", bufs=2)
            nc.scalar.activation(out=pst2, in_=sfg2[:, 0:512], func=Exp, scale=float(SCALE))
            return [h, qh, kv, qlo, qw, nki, pts, pst01, pst2]

        def attention_out(state):
            out_pv(state)
            out_sm(state)
            out_combine(state)

        # ---------------- phase A (+ interleaved attention) --------------------
        with ExitStack() as actx:
            pa = actx.enter_context(tc.tile_pool(name="pa", bufs=1))
            wp = actx.enter_context(tc.tile_pool(name="wp", bufs=2))
            rt = actx.enter_context(tc.tile_pool(name="rt", bufs=2))

            def dma_w(w, dc):
                for q4 in range(4):
                    nc.sync.dma_start(out=w[:, q4 * 8:(q4 + 1) * 8, :],
                                      in_=w1_d[dc, :, q4 * 8:(q4 + 1) * 8, :])

            w0 = wp.tile([128, FCH, 128], BF16, tag="w")
            dma_w(w0, 0)
            hT = pa.tile([128, FCH, S], BF16, tag="hT")
            for fc in range(FCH):
                nc.sync.dma_start(out=hT[:, fc, :], in_=hT_d[fc])
            nc.sync.dma_start(out=cosT, in_=cos_d)
            nc.sync.dma_start(out=sinT, in_=sin_d)

            def rope(ps, dest, sl, qw):
                tcos = rt.tile([128, 512], F32, tag="tcos")
                nc.vector.tensor_mul(tcos[:, 0:qw], ps[:, 0:qw], cosT[:, sl])
                rot = rt.tile([128, 512], F32, tag="rot")
                nc.scalar.copy(rot[0:64, 0:qw], ps[64:128, 0:qw])
                nc.scalar.copy(rot[64:128, 0:qw], ps[0:64, 0:qw])
                tsin = rt.tile([128, 512], F32, tag="tsin")
                nc.vector.tensor_mul(tsin[:, 0:qw], rot[:, 0:qw], sinT[:, sl])
                nc.vector.tensor_add(dest, tcos[:, 0:qw], tsin[:, 0:qw])

            # dc roles: 0,1 = K kv0/kv1; 2,3 = V kv0/kv1; 4..11 = Q h0..h7
            pending = []
            wo0 = None
            for dc in range(DC):
                if dc == 0:
                    w = w0
                else:
                    w = wp.tile([128, FCH, 128], BF16, tag="w")
                    dma_w(w, dc)
                if dc == DC - 1:
                    # prefetch half of the first Wo column block so phase C's
                    # first matmul group isn't DMA-gated
                    wo0 = bp.tile([128, NH // 2, 512], BF16, tag="wo0", bufs=1)
                    for h in range(NH // 2):
                        nc.sync.dma_start(out=wo0[:, h, :], in_=wo_d[h, :, 0:512])
                if dc == 2:
                    # suffix K/V loads deferred so they don't compete with hT
                    # for startup DMA bandwidth (first needed at dc=4)
                    for kv2 in range(NKV):
                        for j in range(LCK):
                            nc.sync.dma_start(out=ksT[:, kv2, j, :], in_=ks_d[kv2, j])
                            nc.sync.dma_start(out=vsT[:, kv2, j, :], in_=vs_d[kv2, j])
                is_q = dc >= 2 * NKV
                for sh in range(2):
                    qw = qwidth(sh) if is_q else 512
                    sl = slice(sh * 512, sh * 512 + qw)
                    ps = pp.tile([128, 512], F32, tag="ps", bufs=2)
                    for fc in range(FCH):
                        nc.tensor.matmul(ps[:, 0:qw], w[:, fc, :], hT[:, fc, sl],
                                         start=(fc == 0), stop=(fc == FCH - 1))
                    if pending:
                        attention_out(pending.pop(0))
                    if not is_q:
                        if dc < NKV:
                            rope(ps, kt[:, dc, sl], sl, 512)
                        else:
                            kv = dc - NKV
                            vstage = rt.tile([128, 512], F32, tag="vstage")
                            nc.vector.tensor_copy(out=vstage, in_=ps)
                            tp = pp.tile([128, 512], F32, tag="ps", bufs=2)
                            for t4 in range(4):
                                nc.tensor.transpose(tp[:, t4 * 128:(t4 + 1) * 128],
                                                    vstage[:, t4 * 128:(t4 + 1) * 128],
                                                    ident)
                            nc.vector.tensor_copy(out=vn[:, kv, sh * 4:(sh + 1) * 4, :], in_=tp)
                    else:
                        h = dc - 2 * NKV
                        rope(ps, qt[:, h, sl], sl, qw)
                        pending.append(attention_scores(h, sh))
            while pending:
                attention_out(pending.pop(0))

        # ---------------- phase C: output projection -----------------------
        with ExitStack() as cctx:
            wp2 = cctx.enter_context(tc.tile_pool(name="wp2", bufs=1))
            for ncol in range(8):
                nsl = slice(ncol * 512, ncol * 512 + 512)
                wo_t = wp2.tile([128, NH, 512], BF16, tag="wo", bufs=3)
                for h in range(NH):
                    if ncol > 0 or h >= NH // 2:
                        nc.sync.dma_start(out=wo_t[:, h, :], in_=wo_d[h, :, nsl])
                for scp in range(4):
                    fo = pp.tile([128, 1024], F32, tag="stg", bufs=2)
                    for half in range(2):
                        sc = scp * 2 + half
                        for h in range(NH):
                            wsrc = wo0[:, h, :] if (ncol == 0 and h < NH // 2) else wo_t[:, h, :]
                            nc.tensor.matmul(fo[:, half * 512:half * 512 + 512],
                                             ot[:, h, sc * 128:(sc + 1) * 128],
                                             wsrc, start=(h == 0), stop=(h == NH - 1))
                        fo_sb = wp2.tile([128, 512], F32, tag="fosb", bufs=4)
                        nc.vector.tensor_copy(out=fo_sb, in_=fo[:, half * 512:half * 512 + 512])
                        nc.sync.dma_start(out=out_d[sc * 128:(sc + 1) * 128, nsl], in_=fo_sb)
    nc.compile()
    return nc


_NC = {}
_LAST_QHI = 1024


def _get_nc(qhi=None):
    global _LAST_QHI
    if qhi is None:
        qhi = _LAST_QHI
    _LAST_QHI = qhi
    if qhi not in _NC:
        _NC[qhi] = _build(qhi)
    return _NC[qhi]


def kernel(hidden_states, k_suffix, v_suffix, Wq, Wk, Wv, Wo, valid_seq_len):
    B = hidden_states.shape[0]
    valid = int(np.asarray(valid_seq_len))
    qhi = valid if 512 < valid < 1024 else 1024

    # rope tables, transposed to [d, s], sin sign-folded for rotate_half
    inv_freq = 1.0 / (10000.0 ** (np.arange(0, D, 2, dtype=np.float32) / D))
    pos = np.arange(S, dtype=np.float32)
    freqs = pos[:, None] * inv_freq[None, :]
    emb = np.concatenate([freqs, freqs], axis=-1)          # [S, D]
    cosT = np.cos(emb).T.astype(np.float32).copy()         # [D, S]
    sinT = np.sin(emb).T.astype(np.float32).copy()
    sgn = np.where(np.arange(D) < D // 2, -1.0, 1.0).astype(np.float32)
    sinT = sinT * sgn[:, None]

    in_maps = []
    for core in range(8):
        b = core // 4
        hg = core % 4
        qsl = slice(hg * NH * D, (hg + 1) * NH * D)
        kvsl = slice(hg * NKV * D, (hg + 1) * NKV * D)

        hT = np.ascontiguousarray(hidden_states[b].T).reshape(FCH, 128, S)
        # dc order: K(2), V(2), Q(8)
        w1 = np.concatenate([Wk[:, kvsl], Wv[:, kvsl], Wq[:, qsl]], axis=1)  # [4096, 1536]
        w1 = w1.reshape(FCH, 128, DC, 128).transpose(2, 1, 0, 3)             # [DC, 128p, FCH, 128m]
        ks = k_suffix[b, hg * NKV:(hg + 1) * NKV].transpose(0, 1, 3, 2)      # [NKV, LCK, 128d, S]
        vs = v_suffix[b, hg * NKV:(hg + 1) * NKV].transpose(0, 1, 3, 2)
        wo = Wo[hg * NH * D:(hg + 1) * NH * D].reshape(NH, 128, HIDDEN)

        in_maps.append({
            "hT": hT.astype(ml_dtypes.bfloat16),
            "w1": np.ascontiguousarray(w1).astype(ml_dtypes.bfloat16),
            "cosT": cosT.astype(ml_dtypes.bfloat16),
            "sinT": sinT.astype(ml_dtypes.bfloat16),
            "ksT": np.ascontiguousarray(ks).astype(ml_dtypes.bfloat16),
            "vsT": np.ascontiguousarray(vs).astype(ml_dtypes.bfloat16),
            "wo": np.ascontiguousarray(wo).astype(ml_dtypes.bfloat16),
        })

    global _LAST_IN_MAPS
    _LAST_IN_MAPS = in_maps
    nc = _get_nc(qhi)
    res = run_bass_kernel_spmd(nc, in_maps, core_ids=list(range(8)))

    out = np.zeros((B, S, HIDDEN), dtype=np.float32)
    for core in range(8):
        out[core // 4] += res.results[core]["out"]
    out[:, valid:, :] = 0.0
    return out


if __name__ == "__main__":
    rng = np.random.default_rng(0)
    h = rng.standard_normal((2, S, HIDDEN)).astype(np.float32)
    ks = rng.standard_normal((2, 8, LCK, S, D)).astype(np.float32)
    vs = rng.standard_normal((2, 8, LCK, S, D)).astype(np.float32)
    wq = (rng.standard_normal((HIDDEN, HIDDEN)) * 0.02).astype(np.float32)
    wk = (rng.standard_normal((HIDDEN, 1024)) * 0.02).astype(np.float32)
    wv = (rng.standard_normal((HIDDEN, 1024)) * 0.02).astype(np.float32)
    wo = (rng.standard_normal((HIDDEN, HIDDEN)) * 0.02).astype(np.float32)
    o = kernel(hidden_states=h, k_suffix=ks, v_suffix=vs, Wq=wq, Wk=wk, Wv=wv, Wo=wo,
               valid_seq_len=960)
    print(o.shape, o.dtype, np.abs(o).max())
